# revision 17
# baseline (speedup 1.0000x reference)
"""Trainium2 Bass kernel for nn_CrossAttention_43061342110469.

Mathematical reduction: the reference's second einsum
    attn = einsum('bvhd,bhqk->bvhd', v, scores)
shares no contraction index with v, so it multiplies v elementwise by
S[b,h] = sum_{q,k} scores[b,h,q,k].  scores is a softmax over k, so every
row sums to 1 and S[b,h] == L == 2048 (verified: the fp32 reference
computes S == 2048.0 bit-exactly; end-to-end rel-err of the reduction is
~5e-7).  Therefore:

    out = (x @ Wv + bv) * 2048 @ Wo + bo
        = x @ (2048 * Wv @ Wo) + (2048 * bv @ Wo + bo)
        = x @ W' + b'

W' (1024x1024) and b' are folded on the host (float64 GEMM, ~ms), so the
device runs a single 8192x1024x1024 GEMM, row-sharded: 1024 rows per
core.  x and W' are cast to bf16 on the host (rel-err of the bf16 path vs
the fp32 reference is ~2.9e-3, tolerance 2e-2); x is pre-transposed
per-shard on the host (matmul wants the contraction dim on partitions for
both operands).

The device computes out^T[dout, row] (lhsT = W' tile, rhs = x^T slice) so
the bias is a per-partition scalar (host-preshaped [128,8] tile, 4 KB;
DVE tensor_scalar and ACT activation-add then both do PSUM copybacks in
parallel).  The host transposes out^T back and upcasts to f32.

Per-core device program:
  - DMA in: x^T [1024d, 1024r] bf16; W' split column-wise into wa
    (douts 0-511, needed first) and wb (douts 512-1023, needed ~15us
    later).  Three explicit per-queue streams in need-order (small chunks
    at the head so round 0 starts early, larger later), depth-3
    completion chains so each queue's descriptor generation pipelines;
    all three queues streaming concurrently reach the ~430 GB/s fabric
    ceiling.
  - ~2.6us of dummy matmuls from t0 so the PE HAM clock-gate opens while
    the first chunks land.
  - Phase A (douts 0-511): ko-outer accumulation over all 8 PSUM banks
    (4 dout-tiles x 2 512-row halves) so contraction step ko runs as
    soon as chunk pair ko lands.
  - Phase B (douts 512-1023): all data resident; K-contiguous per
    dout-tile so output tiles complete early and stream out.  The final
    dout-tile is split into 4 x 256-row sub-groups so the last
    copyback+DMA tail is ~64KB instead of ~256KB.
  - Copybacks alternate DVE (even) and ACT (odd) so freed PSUM banks are
    available ~0.7us after each phase-A stop.
q/k/softmax are numerically dead and not computed.
"""

import sys

import numpy as np

_REPO = "/opt/trn_rl_repo"
if _REPO not in sys.path:
    sys.path.insert(0, _REPO)

B, L, D = 4, 2048, 1024
NCORES = 8
ROWS = B * L  # 8192
R = ROWS // NCORES  # 1024 rows per core
P = 128
NT = 512  # matmul free-dim tile (one PSUM bank of fp32)
KO = D // P  # 8 contraction tiles
JT = D // P  # 8 output-dim tiles
WHALF = D // 2  # 512: wa/wb column split

_NC_CACHE = {}


def build_nc():
    """Build + compile the per-core Bass program (cached)."""
    if "nc" in _NC_CACHE:
        return _NC_CACHE["nc"]

    from contextlib import ExitStack

    import concourse.tile as tile
    from concourse import bacc, mybir
    from concourse.tile_rust import add_dep_helper
    from concourse._compat import get_trn_type

    f32 = mybir.dt.float32
    bf16 = mybir.dt.bfloat16

    nc = bacc.Bacc(
        get_trn_type() or "TRN2",
        target_bir_lowering=False,
        debug=False,
        num_devices=NCORES,
    )

    xt_nd = nc.dram_tensor("xt", [D, R], bf16, kind="ExternalInput").ap()
    wa_nd = nc.dram_tensor("wa", [D, WHALF], bf16, kind="ExternalInput").ap()
    wb_nd = nc.dram_tensor("wb", [D, WHALF], bf16, kind="ExternalInput").ap()
    b2_nd = nc.dram_tensor("b2", [P, JT], f32, kind="ExternalInput").ap()
    out_nd = nc.dram_tensor("out", [D, R], bf16, kind="ExternalOutput").ap()

    with tile.TileContext(nc) as tc, ExitStack() as ctx:
        const = ctx.enter_context(tc.tile_pool(name="const", bufs=1))
        big = ctx.enter_context(tc.tile_pool(name="big", bufs=1))
        psp = ctx.enter_context(tc.tile_pool(name="psp", bufs=8, space="PSUM"))
        outp = ctx.enter_context(tc.tile_pool(name="outp", bufs=4))

        xt_sb = big.tile([P, KO, R], bf16)  # x^T as [d_in, d_out, row]
        wa_sb = big.tile([P, KO, WHALF], bf16)  # W'[:, :512] as [p, ko, n]
        wb_sb = big.tile([P, KO, WHALF], bf16)  # W'[:, 512:] as [p, ko, n]
        b2 = const.tile([P, JT], f32)  # b2[p, j] = b'[j*128 + p]

        # --- PE warmup first in program order: dummy matmuls from t0 so
        # the HAM clock-gate opens (K=8/8 @ 2.4GHz) as real work arrives.
        # DVE memset: the DVE stream is otherwise empty until copybacks, so
        # it issues right at engine start (gpsimd would queue it behind the
        # chained DMA waits).
        warm = const.tile([P, P], bf16)
        nc.vector.memset(warm[:], 1.0)
        wps = psp.tile([P, NT], f32, tag="t", name="wps")
        for _ in range(28):
            nc.tensor.matmul(
                wps[:, 0:P], lhsT=warm[:], rhs=warm[:], start=True, stop=True
            )

        xt_r = xt_nd.rearrange("(ko p) r -> p ko r", p=P)
        wa_r = wa_nd.rearrange("(ko p) n -> p ko n", p=P)
        wb_r = wb_nd.rearrange("(ko p) n -> p ko n", p=P)

        # --- DMA schedule: three explicit per-queue streams in need-order.
        # Phase-A round ko needs (wa[ko], xt[ko]); heads are 128KB so round
        # 0 can start ~1us earlier; wb and b2 are needed only ~15us in.
        qjobs = [
            # sync queue: early xt row-half chunks so n=0 half-rounds start
            # on 128KB arrivals, then later xt chunks, wb last
            [
                (xt_sb[:, 0, 0:NT], xt_r[:, 0, 0:NT]),
                (xt_sb[:, 1, 0:NT], xt_r[:, 1, 0:NT]),
                (xt_sb[:, 2, 0:NT], xt_r[:, 2, 0:NT]),
                (xt_sb[:, 5], xt_r[:, 5]),
                (xt_sb[:, 6], xt_r[:, 6]),
                (wb_sb[:, 0:4], wb_r[:, 0:4]),
            ],
            # scalar queue
            [
                (wa_sb[:, 0], wa_r[:, 0]),
                (xt_sb[:, 1, NT:R], xt_r[:, 1, NT:R]),
                (xt_sb[:, 3], xt_r[:, 3]),
                (xt_sb[:, 7], xt_r[:, 7]),
                (wb_sb[:, 4:8], wb_r[:, 4:8]),
            ],
            # gpsimd queue
            [
                (xt_sb[:, 0, NT:R], xt_r[:, 0, NT:R]),
                (wa_sb[:, 1], wa_r[:, 1]),
                (wa_sb[:, 2], wa_r[:, 2]),
                (wa_sb[:, 3], wa_r[:, 3]),
                (xt_sb[:, 2, NT:R], xt_r[:, 2, NT:R]),
                (xt_sb[:, 4], xt_r[:, 4]),
                (wa_sb[:, 4:6], wa_r[:, 4:6]),
                (wa_sb[:, 6:8], wa_r[:, 6:8]),
                (b2[:], b2_nd),
            ],
        ]

        qs = [nc.sync, nc.scalar, nc.gpsimd]
        chains = [[], [], []]

        def chained_dma(qi, dst, srcap):
            inst = qs[qi].dma_start(dst, srcap)
            ch = chains[qi]
            if len(ch) >= 4:
                add_dep_helper(inst.ins, ch[-4].ins, sync=True, reason="dma chain")
            ch.append(inst)
            return inst

        for qi, jobs in enumerate(qjobs):
            for dst, srcap in jobs:
                chained_dma(qi, dst, srcap)

        outs = {}

        def copyback(j, lo, hi, ps, engine, qi=None):
            if j not in outs:
                outs[j] = outp.tile([P, R], bf16, name=f"ot{j}")
            ot = outs[j]
            if engine == 0:
                nc.vector.tensor_scalar_add(ot[:, lo:hi], ps[:, 0 : hi - lo], b2[:, j : j + 1])
            else:
                nc.scalar.add(ot[:, lo:hi], ps[:, 0 : hi - lo], b2[:, j : j + 1])
            chained_dma(
                (j + lo // NT) % 3 if qi is None else qi,
                out_nd[j * P : (j + 1) * P, lo:hi],
                ot[:, lo:hi],
            )

        # Phase A: douts 0-511, ko-outer across all 8 PSUM banks; step ko
        # fires as soon as DMA pair ko lands.
        pssA = {
            (j, n): psp.tile([P, NT], f32, tag="t", name=f"gA_{j}_{n}")
            for j in range(4)
            for n in range(2)
        }
        for ko in range(KO):
            for n in range(2):  # n-major: the n=0 half-round only needs the
                for j in range(4):  # first 128KB half of xt[ko]
                    nc.tensor.matmul(
                        pssA[(j, n)][:],
                        lhsT=wa_sb[:, ko, j * P : (j + 1) * P],
                        rhs=xt_sb[:, ko, n * NT : (n + 1) * NT],
                        start=(ko == 0),
                        stop=(ko == KO - 1),
                    )
        # copy banks in the order phase B will want them back
        for j in range(4):
            for n in range(2):
                copyback(j, n * NT, (n + 1) * NT, pssA[(j, n)], n % 2)

        # Phase B: douts 512-895, K-contiguous per dout-tile (all data
        # resident); output tiles complete early and stream out.
        for j in range(4, 7):
            pss = [psp.tile([P, NT], f32, tag="t", name=f"gB_{j}_{n}") for n in range(2)]
            for ko in range(KO):
                for n in range(2):
                    nc.tensor.matmul(
                        pss[n][:],
                        lhsT=wb_sb[:, ko, (j - 4) * P : (j - 3) * P],
                        rhs=xt_sb[:, ko, n * NT : (n + 1) * NT],
                        start=(ko == 0),
                        stop=(ko == KO - 1),
                    )
            for n in range(2):
                copyback(j, n * NT, (n + 1) * NT, pss[n], n % 2)

        # Final dout-tile: 4 x 256-row K-contiguous sub-groups so the last
        # copyback+DMA after the final matmul is only ~64KB.
        j = 7
        for q in range(4):
            ps = psp.tile([P, NT], f32, tag="t", name=f"gB7_{q}")
            for ko in range(KO):
                nc.tensor.matmul(
                    ps[:, 0:256],
                    lhsT=wb_sb[:, ko, 3 * P : 4 * P],
                    rhs=xt_sb[:, ko, q * 256 : (q + 1) * 256],
                    start=(ko == 0),
                    stop=(ko == KO - 1),
                )
            copyback(j, q * 256, (q + 1) * 256, ps, q % 2, qi=q % 2)

    nc.compile()
    _NC_CACHE["nc"] = nc
    return nc


def make_in_maps(inputs):
    import ml_dtypes

    bf16 = ml_dtypes.bfloat16

    wv = np.asarray(inputs["Wv"], dtype=np.float64)
    bv = np.asarray(inputs["bv"], dtype=np.float64)
    wo = np.asarray(inputs["Wo"], dtype=np.float64)
    bo = np.asarray(inputs["bo"], dtype=np.float64)
    wf = (2048.0 * (wv @ wo)).astype(np.float32).astype(bf16)
    bf = ((2048.0 * (bv @ wo)) + bo).astype(np.float32)

    xf = np.asarray(inputs["x"], dtype=np.float32).reshape(ROWS, D).astype(bf16)
    wa = np.ascontiguousarray(wf[:, :WHALF])
    wb = np.ascontiguousarray(wf[:, WHALF:])
    b2 = np.ascontiguousarray(bf.reshape(JT, P).T)  # b2[p, j] = b'[j*128+p]
    return [
        {
            "xt": np.ascontiguousarray(xf[c * R : (c + 1) * R].T),
            "wa": wa,
            "wb": wb,
            "b2": b2,
        }
        for c in range(NCORES)
    ]


def kernel(**inputs) -> np.ndarray:
    from concourse.bass_utils import run_bass_kernel_spmd

    nc = build_nc()
    in_maps = make_in_maps(inputs)
    res = run_bass_kernel_spmd(nc, in_maps, list(range(NCORES)))
    out = np.concatenate(
        [np.asarray(res.results[c]["out"]).T.astype(np.float32) for c in range(NCORES)],
        axis=0,
    ).reshape(B, L, D)
    return np.ascontiguousarray(out)
